# revision 32
# baseline (speedup 1.0000x reference)
"""Trainium2 Bass kernel for a 2-layer message-passing GNN (BaselineGNN).

Reference computation (N=4096 nodes, IN=512, HID=4096, E=65536 edges):
    h   = x @ We.T + be                                   [N, HID]
    for W, b in ((W1, b1), (W2, b2)):
        aggr = segment_sum(h[col], row)                   [N, HID]
        h    = relu(aggr @ W.T + b)
    hm  = mean(h, axis=1)                                 [N]
    z   = relu(hm @ Wc1.T + bc1)                          [HID//2]
    out = (z @ Wc2.T + bc2).squeeze(-1)                   scalar

Strategy (8 NeuronCores, node-parallel):
  * segment_sum == A @ h with A the [N, N] adjacency-count matrix (0.4%
    dense).  A's entries are small integer counts -> exactly representable
    in fp8-e4m3, so aggregation runs as a dense TensorEngine matmul.
  * Nodes are sharded: core c owns rows 512c..512c+512.  Weight matmuls
    are then fully local; each core computes A_c @ h with A_c = A[rows_c].
  * Layer 1 is low-rank through the embed bottleneck and collapses to
        h1_c = relu((A_c @ x_ext) @ (We_ext.T @ W1.T) + b1)
    with the weight product folded on the host; x_ext carries an extra
    all-ones column and We_ext.T an extra b_embed row.  b1 itself is
    folded the same way: tT carries an all-ones contraction row whose
    matching wcb row holds b1.
  * All four big matmuls run fp8 DoubleRow.  wcb is pre-scaled by CBS so
    its 0.009-sigma entries clear e4m3's subnormal range; the layer-1
    PSUM eviction divides CBS back out inside the relu (activation
    scale).  W2 is pre-scaled by WSCALE=64 (divided out of hm at the
    mean-pool).  b2 is folded as an extra ones-row of the M5 contraction.
  * Pipeline per core: M1 tT=(A_c@x_ext).T -> M3 h1_c (manual loop,
    column-quarter-major) -> 4 quarter AllGathers fired as each quarter
    completes (overlap M3/M4) -> M4 a2T=(A_c@h1).T (one composable
    matmul spanning the 4 gathered quarters) -> M5 h2=relu(a2T.T@w2)
    (manual loop, w2 double-buffered in SBUF quarters, relu+row-sum
    fused into the eviction) -> local partial z = Wc1@hm -> AllReduce z
    -> epilogue -> scalar.
  * The M3/M5 manual loops cycle PSUM bank groups so evictions never
    stall the PE; all PSUM readers are pinned to the scalar engine
    (vector-engine PSUM reads and the HW accumulator were both measured
    to slow concurrent PE matmuls ~15%).
  * DMA queue order is managed so latency-critical transfers (h1c
    writes feeding AllGather triggers, M4's h1f k-tiles) never sit
    behind bulk weight prefetches: wcb half 1 and the w2/wc1 prefetch
    ride behind the AllGather triggers, and M1/M4 pull their operands
    in via producer callbacks with deep tile-pool buffering.
"""

import contextlib

import numpy as np
import ml_dtypes

import concourse.bass as bass
import concourse.mybir as mybir
import concourse.tile as tile
from concourse import bacc
from concourse.bass import ds, ts
from concourse.bass_interp import get_hw_module
from concourse.bass_utils import run_bass_kernel_spmd
from concourse.kernels.tile_matmul import (
    ShapeInfo,
    composable_matmul_tile_kernel,
    dma_from_dram_kxm,
    dma_from_dram_kxn,
    scalar_copyback,
)

N = 4096          # nodes
IN_DIM = 512
HID = 4096
NCORES = 8
S = N // NCORES           # nodes per core (512)
KE = 640                  # extended embed contraction (512 + 1 ones col, padded to 5*128)
KES = 6                   # M3 contraction subtiles: KE rows + ones/b1 row, padded to 768
CHID = HID // 2           # classifier hidden (2048)
MSUB = S // 128           # 4 m-subtiles per 512-node core slice
NAG = 2                   # h1 column halves, one AllGather each
HQ = HID // NAG           # 2048
WQ = HID // 4             # 1024: M5 w2 ring quarter
K2S = 34                  # M5 contraction subtiles: HID + ones/b2 row, padded to 4352

BF16 = mybir.dt.bfloat16
F32 = mybir.dt.float32
FP8 = mybir.dt.float8e4
ADT = FP8

WSCALE = 64.0             # W2 pre-scale (divided out of hm)
CBS = 2048.0              # wcb pre-scale (divided out in the M3 eviction)

_COMPILED = {}


def _build_graph(use_b2=False):
    nc = bacc.Bacc(
        "TRN2",
        target_bir_lowering=False,
        debug=False,
        enable_asserts=False,
        num_devices=NCORES,
    )

    # ---- kernel I/O (per core) ----
    xe = nc.dram_tensor("xe", [N, KE], ADT, kind="ExternalInput")           # x_ext (replicated)
    at8 = nc.dram_tensor("at8", [N, S], ADT, kind="ExternalInput")          # A.T[:, rows_c] (sharded)
    wcb = nc.dram_tensor("wcb", [KES * 128, HID], ADT, kind="ExternalInput")  # CBS*(We_ext.T W1.T; b1) (repl)
    w2 = nc.dram_tensor("w2", [K2S * 128, HID], ADT, kind="ExternalInput")  # WSCALE*(W2.T; b2) (repl)
    wc1 = nc.dram_tensor("wc1", [S, CHID], BF16, kind="ExternalInput")      # Wc1.T row-chunk (sharded)
    bc1 = nc.dram_tensor("bc1", [128, CHID // 128], F32, kind="ExternalInput")  # bc1 [128,16]
    wc2 = nc.dram_tensor("wc2", [128, CHID // 128], F32, kind="ExternalInput")  # Wc2 [128,16]
    res = nc.dram_tensor("res", [1, 1], F32, kind="ExternalOutput")         # final scalar (pre-bc2)

    # ---- internal DRAM ----
    h1c_q = [nc.dram_tensor(f"h1c{i}", [S, HQ], ADT) for i in range(NAG)]
    h1f_q = [
        nc.dram_tensor(f"h1f{i}", [N, HQ], ADT, addr_space="Shared")
        for i in range(NAG)
    ]
    zb = nc.dram_tensor("zb", [1, CHID], F32)           # local partial Wc1 @ hm
    zf = nc.dram_tensor("zf", [1, CHID], F32, addr_space="Shared")  # allreduced

    NT = HID // 512   # 8 n-tiles of 512
    CI = CHID // 128  # 16
    DR = mybir.MatmulPerfMode.DoubleRow
    Relu = mybir.ActivationFunctionType.Relu
    Copy = mybir.ActivationFunctionType.Copy
    groups = [list(range(NCORES))]

    def scalar_copy_reducer(nc_, psum, sbuf, md):
        # pinned to the scalar engine: 'any'-assigned PSUM readers sometimes
        # land on the vector engine, which was measured to slow concurrent
        # PE matmuls by ~15%
        nc_.scalar.activation(out=sbuf, in_=psum, func=Copy)

    w2v = w2[:, :].rearrange("(po pi) n -> pi po n", pi=128)
    h1fv = [h1f_q[i][:, :].rearrange("(po pi) n -> pi po n", pi=128) for i in range(NAG)]

    with tile.TileContext(nc) as tc:
        with contextlib.ExitStack() as octx:
            const = octx.enter_context(tc.tile_pool(name="const", bufs=1))
            hm_parts = const.tile([128, MSUB, NT], F32, name="hm_parts")
            hm_f32 = const.tile([128, MSUB], F32, name="hm_f32")
            hm16 = const.tile([128, MSUB], BF16, name="hm16")

            head = octx.enter_context(tc.tile_pool(name="head", bufs=1))
            wc1_t = head.tile([128, MSUB, CHID], BF16, name="wc1_t")
            bc1_t = head.tile([128, CI], F32, name="bc1_t")
            wc2_t = head.tile([128, CI], F32, name="wc2_t")
            zp_t = head.tile([1, CHID], F32, name="zp_t")
            z2_t = head.tile([128, CI], F32, name="z2_t")
            zcol_t = head.tile([128, 1], F32, name="zcol_t")
            ones_t = head.tile([128, 1], F32, name="ones_t")
            r_t = head.tile([1, 1], F32, name="r_t")
            zero_t = head.tile([128, 512], F32, name="zero_t")
            nc.any.memset(ones_t[:], 1.0)
            nc.any.memset(zero_t[:], 0.0)

            # persistent SBUF caches for chained-matmul intermediates
            cache = octx.enter_context(tc.tile_pool(name="cache", bufs=1))
            at8_c = cache.tile([128, N // 128, S], ADT, name="at8_c")
            tT_c = cache.tile([128, KES, S], ADT, name="tT_c")
            wcb_c = cache.tile([128, KES, HID], ADT, name="wcb_c")
            a2T_c = cache.tile([128, K2S, S], ADT, name="a2T_c")
            # w2 ring: 2 quarters x 2 halves as separate 512-wide tiles so
            # the matmul rhs pattern is stride-512 (measured ~15% faster on
            # the PE than 2048-wide strided reads)
            w2t = [cache.tile([128, K2S, 512], ADT, name=f"w2t{i}") for i in range(4)]

            nc.any.memset(hm_parts[:], 0.0)
            # ones/bias contraction rows: tT row 640 multiplies wcb's b1 row;
            # a2T row 4096 multiplies w2's b2 row (only when b2 is nonzero)
            nc.any.memset(tT_c[:, KES - 1 :, :], 0.0)
            nc.any.memset(tT_c[0:1, KES - 1 :, :], 1.0)
            nc.any.memset(a2T_c[:, 32:, :], 0.0)
            if use_b2:
                nc.any.memset(a2T_c[0:1, 32:33, :], 1.0)

            # at8 cache: chunk 0 up front; M1's kxn producer pulls in the
            # rest per k-tile so M1's xe stream is never queued behind it
            at8v = at8[:, :].rearrange("(po pi) n -> pi po n", pi=128)
            nc.sync.dma_start(out=at8_c[:, 0:8, :], in_=at8v[:, 0:8, :])
            wcbv = wcb[:, :].rearrange("(po pi) n -> pi po n", pi=128)

            noop = lambda nc_, sbuf, md: None

            # M1: tT = (A_c @ x_ext).T = xe.T @ A_c.T        [KE, S] fp8
            m1_loaded = {0}

            def m1_kxn(nc_, md):
                kt = md.k_tile_idx
                if kt not in m1_loaded:
                    m1_loaded.add(kt)
                    nc_.sync.dma_start(
                        out=at8_c[:, ts(kt, md.k_subtiles), :],
                        in_=at8v[:, ts(kt, md.k_subtiles), :],
                    )
                return at8_c[:, ts(kt, md.k_subtiles), :]

            with contextlib.ExitStack() as ctx:
                kxm_pool = ctx.enter_context(tc.tile_pool(name="m1kxm", bufs=10))
                kxm_producer, kxm_shape = dma_from_dram_kxm(kxm_pool, xe[:, :])
                composable_matmul_tile_kernel(
                    tc=tc,
                    kxm_shape=kxm_shape,
                    kxn_shape=ShapeInfo(pdims=((128, N // 128),), fdims=(S,)),
                    output_type=None,
                    kxm_producer=kxm_producer,
                    kxn_producer=m1_kxn,
                    mxn_subtile_reducer=scalar_copy_reducer,
                    mxn_consumer=noop,
                    mxn_subtile_producer=lambda nc_, md: tT_c[
                        :, md.m_tile_idx : md.m_tile_idx + 1, :
                    ],
                    psum_n_bufs=2,
                    MAX_K_TILE_SIZE=1024,
                )
            # wcb half 0 streams in during M1's compute; half 1 is issued
            # inside the M3 loop behind the first AllGather trigger
            nc.sync.dma_start(out=wcb_c[:, :, 0:HQ], in_=wcbv[:, :, 0:HQ])

            # M3: h1 = relu((tT.T @ wcb) / CBS)   [S, HID] fp8, manual loop.
            # Column-quarter-major so each AllGather fires as soon as its
            # quarter is complete; each DoubleRow stationary feeds 2 PSUM
            # banks; 4 bank-pairs cycle so evictions never stall the PE.
            with contextlib.ExitStack() as ctx:
                m3psum = ctx.enter_context(
                    tc.tile_pool(name="m3psum", bufs=1, space="PSUM")
                )
                m3ps = [
                    [m3psum.tile([128, 512], F32, name=f"m3ps{p}_{b}") for b in range(4)]
                    for p in range(2)
                ]
                m3out = ctx.enter_context(tc.tile_pool(name="m3out", bufs=4))
                for q in range(NAG):
                    for ms in range(MSUB):
                        grp = m3ps[(q * MSUB + ms) % 2]
                        for kk in range(KES // 2):
                            lhsT = tT_c[:, 2 * kk : 2 * kk + 2, 128 * ms : 128 * (ms + 1)]
                            for b in range(4):
                                nc.tensor.matmul(
                                    grp[b][:, :],
                                    lhsT,
                                    wcb_c[:, 2 * kk : 2 * kk + 2,
                                          HQ * q + 512 * b : HQ * q + 512 * (b + 1)],
                                    start=(kk == 0),
                                    stop=(kk == KES // 2 - 1),
                                    perf_mode=DR,
                                )
                        h1t = m3out.tile([128, HQ], ADT, tag="h1t")
                        for b in range(4):
                            # alternate scalar/vector so neither engine lags
                            if b % 2 == 0:
                                nc.scalar.activation(
                                    out=h1t[:, 512 * b : 512 * (b + 1)],
                                    in_=grp[b][:, :], func=Relu, scale=1.0 / CBS,
                                )
                            else:
                                nc.vector.tensor_scalar(
                                    h1t[:, 512 * b : 512 * (b + 1)], grp[b][:, :],
                                    1.0 / CBS, 0.0,
                                    mybir.AluOpType.mult, mybir.AluOpType.max,
                                )
                        nc.sync.dma_start(
                            out=h1c_q[q][128 * ms : 128 * (ms + 1), :], in_=h1t[:, :]
                        )
                    nc.gpsimd.collective_compute(
                        "AllGather",
                        mybir.AluOpType.bypass,
                        ins=[h1c_q[q][:, :].opt()],
                        outs=[h1f_q[q][:, :].opt()],
                        replica_groups=groups,
                    )
                    if q == 0:
                        # wcb half 1 rides behind half 0's h1c writes so the
                        # first AllGather trigger isn't queued behind it
                        nc.sync.dma_start(
                            out=wcb_c[:, :, HQ : 2 * HQ], in_=wcbv[:, :, HQ : 2 * HQ]
                        )

            # prefetches for M5/M6 fill the DMA-idle window while the PE waits
            # on the first AllGather: w2 quarter 0, then the head weights
            for b in range(2):
                nc.sync.dma_start(
                    out=w2t[b][:, :, :], in_=w2v[:, :, 512 * b : 512 * (b + 1)]
                )
            nc.sync.dma_start(
                out=wc1_t[:, :, :],
                in_=wc1[:, :].rearrange("(po pi) n -> pi po n", pi=128),
            )
            nc.sync.dma_start(out=bc1_t[:, :], in_=bc1[:, :])
            nc.sync.dma_start(out=wc2_t[:, :], in_=wc2[:, :])

            # M4: a2T = (A_c @ h1).T = h1f.T @ A_c.T         [HID, S] fp8.
            # One composable matmul over all of HID; the kxm producer sources
            # each m-tile from its gathered quarter tensor.
            with contextlib.ExitStack() as ctx:
                m4pool = ctx.enter_context(tc.tile_pool(name="m4kxm", bufs=8))

                def m4_kxm(nc_, md):
                    t = m4pool.tile([128, md.k_subtiles, 512], ADT, tag="m4kxm")
                    q, loc = divmod(md.m_tile_idx, HQ // 512)
                    nc_.sync.dma_start(
                        t[:],
                        h1fv[q][:, ts(md.k_tile_idx, md.k_subtiles), ds(loc * 512, 512)],
                    )
                    return t

                kxn_producer, kxn_shape = dma_from_dram_kxn(
                    None, at8_c[:, :, :], kxn_cache=at8_c[:, :, :]
                )
                composable_matmul_tile_kernel(
                    tc=tc,
                    kxm_shape=ShapeInfo(pdims=((128, N // 128),), fdims=(HID,)),
                    kxn_shape=kxn_shape,
                    output_type=None,
                    kxm_producer=m4_kxm,
                    kxn_producer=kxn_producer,
                    mxn_subtile_reducer=scalar_copy_reducer,
                    mxn_consumer=noop,
                    mxn_subtile_producer=lambda nc_, md: a2T_c[
                        :, 4 * md.m_tile_idx : 4 * md.m_tile_idx + 4, :
                    ],
                    psum_n_bufs=2,
                    MAX_K_TILE_SIZE=1024,
                )

            # M5: h2 = relu(a2T.T @ w2); only row-sums survive (-> hm_parts).
            # Manual loop: w2 streams through a 2-quarter SBUF ring, each
            # DoubleRow stationary feeds 2 banks, relu+row-sum fused into the
            # scalar-engine eviction.  b2 rides as contraction row 4096.
            KP = K2S // 2 if use_b2 else 16
            with contextlib.ExitStack() as ctx:
                m5psum = ctx.enter_context(
                    tc.tile_pool(name="m5psum", bufs=1, space="PSUM")
                )
                m5ps = [
                    [m5psum.tile([128, 512], F32, name=f"m5ps{p}_{b}") for b in range(2)]
                    for p in range(4)
                ]
                m5scr = ctx.enter_context(tc.tile_pool(name="m5scr", bufs=2))
                for q in range(4):
                    if q + 1 < 4:  # double-buffer the next w2 quarter
                        for b in range(2):
                            sl_n = 2 * ((q + 1) % 2) + b
                            nc.sync.dma_start(
                                out=w2t[sl_n][:, :, :],
                                in_=w2v[:, :, WQ * (q + 1) + 512 * b :
                                        WQ * (q + 1) + 512 * (b + 1)],
                            )
                    for ms in range(MSUB):
                        pair = m5ps[(q * MSUB + ms) % 4]
                        for b in range(2):
                            rhs_t = w2t[2 * (q % 2) + b]
                            for kk in range(KP):
                                nc.tensor.matmul(
                                    pair[b][:, :],
                                    a2T_c[:, 2 * kk : 2 * kk + 2,
                                          128 * ms : 128 * (ms + 1)],
                                    rhs_t[:, 2 * kk : 2 * kk + 2, :],
                                    start=(kk == 0),
                                    stop=(kk == KP - 1),
                                    perf_mode=DR,
                                )
                            # evict via scalar relu into SBUF, then row-sum on
                            # the vector engine from SBUF: keeps both the HW
                            # accumulator and vector-engine PSUM reads out of
                            # the PE's way (each was measured to slow
                            # concurrent matmuls by ~15%)
                            scr = m5scr.tile([128, 512], F32, tag="m5scr")
                            nc.scalar.activation(
                                out=scr[:, :], in_=pair[b][:, :], func=Relu
                            )
                            nc.vector.tensor_reduce(
                                out=hm_parts[:, ms, 2 * q + b : 2 * q + b + 1],
                                in_=scr[:, :],
                                axis=mybir.AxisListType.X, op=mybir.AluOpType.add,
                            )

            # hm_c = rowsum(h2_c) / (HID * WSCALE), kept in SBUF (bf16 for M6)
            nc.vector.tensor_reduce(
                out=hm_f32[:, :], in_=hm_parts[:, :, :],
                axis=mybir.AxisListType.X, op=mybir.AluOpType.add,
            )
            nc.vector.tensor_scalar_mul(hm16[:, :], hm_f32[:, :], 1.0 / (HID * WSCALE))

            # M6 (head): zp = Wc1[:, local] @ hm_local, AllReduce over cores,
            # then z = relu(z + bc1), res = z . Wc2.  Bank-interleaved so the
            # 16 tiny matmuls pipeline instead of serializing per bank.
            hpsum = octx.enter_context(tc.tile_pool(name="hpsum", bufs=1, space="PSUM"))
            NB = CHID // 512  # 4 psum banks
            ps6 = [hpsum.tile([128, 512], F32, name=f"ps{j}") for j in range(NB)]
            psr = hpsum.tile([128, 512], F32, name="psr")
            for ko in range(MSUB):
                for j in range(NB):
                    nc.tensor.matmul(
                        ps6[j][0:1, :],
                        hm16[:, ko : ko + 1],
                        wc1_t[:, ko, 512 * j : 512 * (j + 1)],
                        start=(ko == 0),
                        stop=(ko == MSUB - 1),
                    )
            for j in range(NB):
                nc.vector.tensor_copy(
                    out=zp_t[:, 512 * j : 512 * (j + 1)], in_=ps6[j][0:1, :]
                )
            nc.sync.dma_start(out=zb[:, :], in_=zp_t[:, :])
            nc.gpsimd.collective_compute(
                "AllReduce",
                mybir.AluOpType.add,
                ins=[zb[:, :].opt()],
                outs=[zf[:, :].opt()],
                replica_groups=groups,
            )
            # epilogue on z viewed as [128, 16] so the DVE ops use all lanes
            nc.sync.dma_start(
                out=z2_t[:, :], in_=zf[:, :].rearrange("o (p i) -> p (o i)", p=128)
            )
            nc.vector.tensor_add(out=z2_t[:, :], in0=z2_t[:, :], in1=bc1_t[:, :])
            # fused relu * wc2 with row-sum: out = max(z2, 0) * wc2,
            # zcol = sum(out) — one DVE op instead of three
            nc.vector.scalar_tensor_tensor(
                out=z2_t[:, :], in0=z2_t[:, :], scalar=0.0, in1=wc2_t[:, :],
                op0=mybir.AluOpType.max, op1=mybir.AluOpType.mult,
                accum_out=zcol_t[:, :],
            )
            # cross-partition sum via a 128x1 ones matmul
            nc.tensor.matmul(
                psr[0:1, 0:1], ones_t[:, 0:1], zcol_t[:, 0:1], start=True, stop=True
            )
            nc.vector.tensor_copy(out=r_t[:, :], in_=psr[0:1, 0:1])
            nc.sync.dma_start(out=res[:, :], in_=r_t[:, :])

    nc.compile()
    nc.m = get_hw_module(nc.m)
    return nc


def get_compiled(use_b2=False):
    key = bool(use_b2)
    if key not in _COMPILED:
        _COMPILED[key] = _build_graph(key)
    return _COMPILED[key]


def _f32(a):
    return np.ascontiguousarray(np.asarray(a, dtype=np.float32))


_NP_FP8 = mybir.dt.np(FP8)


def _adt(a):
    a = np.ascontiguousarray(np.asarray(a, dtype=np.float32))
    return np.clip(a, -240.0, 240.0).astype(_NP_FP8)


def make_in_maps(x, edge_index, W_embed, b_embed, W1, b1, W2, b2, Wc1, bc1, Wc2, bc2):
    x = _f32(x)
    ei = np.asarray(edge_index).astype(np.int64)
    # adjacency counts, transposed: AT[src, dst] = #edges src->dst
    counts = np.bincount(ei[1] * N + ei[0], minlength=N * N).astype(np.float32)
    AT = counts.reshape(N, N)

    x_ext = np.zeros((N, KE), np.float32)
    x_ext[:, :IN_DIM] = x
    x_ext[:, IN_DIM] = 1.0

    we_ext = np.zeros((KE, HID), np.float32)
    we_ext[:IN_DIM] = _f32(W_embed).T
    we_ext[IN_DIM] = _f32(b_embed)
    # layer-1 transform is low-rank: fold We_ext.T @ W1.T on the host.
    # Row 640 holds b1, matched by the all-ones tT contraction row.
    wcb_full = np.zeros((KES * 128, HID), np.float32)
    wcb_full[:KE] = we_ext @ _f32(W1).T
    wcb_full[KE] = _f32(b1)
    wcb_np = _adt(wcb_full * CBS)

    w2p = np.zeros((K2S * 128, HID), np.float32)
    w2p[:HID] = _f32(W2).T * WSCALE
    w2p[HID] = _f32(b2) * WSCALE
    w2_np = _adt(w2p)

    xe_np = _adt(x_ext)
    at8_np = _adt(AT)
    wc1T = _f32(Wc1).T.astype(ml_dtypes.bfloat16)  # [HID(nodes), CHID]
    wc2_row = _f32(Wc2).reshape(128, CHID // 128)
    bc1_full = _f32(bc1).reshape(128, CHID // 128)

    in_maps = []
    for c in range(NCORES):
        rows = slice(S * c, S * (c + 1))
        in_maps.append(
            {
                "xe": xe_np,
                "wcb": wcb_np,
                "at8": np.ascontiguousarray(at8_np[:, rows]),
                "w2": w2_np,
                "wc1": np.ascontiguousarray(wc1T[rows, :]),
                "bc1": bc1_full,
                "wc2": wc2_row,
            }
        )
    return in_maps


def kernel(**inputs):
    use_b2 = bool(np.any(np.asarray(inputs["b2"], dtype=np.float32)))
    nc = get_compiled(use_b2)
    in_maps = make_in_maps(**inputs)
    bres = run_bass_kernel_spmd(nc, in_maps, core_ids=list(range(NCORES)))
    val = np.float32(bres.results[0]["res"][0, 0])
    bc2 = np.asarray(inputs["bc2"], dtype=np.float32).reshape(-1)
    out = np.asarray(val + bc2[0], dtype=np.float32).reshape(())
    return out


# revision 33
# speedup vs baseline: 1.0145x; 1.0145x over previous
"""Trainium2 Bass kernel for a 2-layer message-passing GNN (BaselineGNN).

Reference computation (N=4096 nodes, IN=512, HID=4096, E=65536 edges):
    h   = x @ We.T + be                                   [N, HID]
    for W, b in ((W1, b1), (W2, b2)):
        aggr = segment_sum(h[col], row)                   [N, HID]
        h    = relu(aggr @ W.T + b)
    hm  = mean(h, axis=1)                                 [N]
    z   = relu(hm @ Wc1.T + bc1)                          [HID//2]
    out = (z @ Wc2.T + bc2).squeeze(-1)                   scalar

Strategy (8 NeuronCores, node-parallel):
  * segment_sum == A @ h with A the [N, N] adjacency-count matrix (0.4%
    dense).  A's entries are small integer counts -> exactly representable
    in fp8-e4m3, so aggregation runs as a dense TensorEngine matmul.
  * Nodes are sharded: core c owns rows 512c..512c+512.  Weight matmuls
    are then fully local; each core computes A_c @ h with A_c = A[rows_c].
  * Layer 1 is low-rank through the embed bottleneck and collapses to
        h1_c = relu((A_c @ x_ext) @ (We_ext.T @ W1.T) + b1)
    with the weight product folded on the host; x_ext carries an extra
    all-ones column and We_ext.T an extra b_embed row.  b1 itself is
    folded the same way: tT carries an all-ones contraction row whose
    matching wcb row holds b1.
  * All four big matmuls run fp8 DoubleRow.  wcb is pre-scaled by CBS so
    its 0.009-sigma entries clear e4m3's subnormal range; the layer-1
    PSUM eviction divides CBS back out inside the relu (activation
    scale).  W2 is pre-scaled by WSCALE=64 (divided out of hm at the
    mean-pool).  b2 is folded as an extra ones-row of the M5 contraction.
  * Pipeline per core: M1 tT=(A_c@x_ext).T -> M3 h1_c (manual loop,
    column-quarter-major) -> 4 quarter AllGathers fired as each quarter
    completes (overlap M3/M4) -> M4 a2T=(A_c@h1).T (one composable
    matmul spanning the 4 gathered quarters) -> M5 h2=relu(a2T.T@w2)
    (manual loop, w2 double-buffered in SBUF quarters, relu+row-sum
    fused into the eviction) -> local partial z = Wc1@hm -> AllReduce z
    -> epilogue -> scalar.
  * The M3/M5 manual loops cycle PSUM bank groups so evictions never
    stall the PE; all PSUM readers are pinned to the scalar engine
    (vector-engine PSUM reads and the HW accumulator were both measured
    to slow concurrent PE matmuls ~15%).
  * DMA queue order is managed so latency-critical transfers (h1c
    writes feeding AllGather triggers, M4's h1f k-tiles) never sit
    behind bulk weight prefetches: wcb half 1 and the w2/wc1 prefetch
    ride behind the AllGather triggers, and M1/M4 pull their operands
    in via producer callbacks with deep tile-pool buffering.
"""

import contextlib

import numpy as np
import ml_dtypes

import concourse.bass as bass
import concourse.mybir as mybir
import concourse.tile as tile
from concourse import bacc
from concourse.bass import ds, ts
from concourse.bass_interp import get_hw_module
from concourse.bass_utils import run_bass_kernel_spmd
from concourse.kernels.tile_matmul import (
    ShapeInfo,
    composable_matmul_tile_kernel,
    dma_from_dram_kxm,
    dma_from_dram_kxn,
    scalar_copyback,
)

N = 4096          # nodes
IN_DIM = 512
HID = 4096
NCORES = 8
S = N // NCORES           # nodes per core (512)
KE = 640                  # extended embed contraction (512 + 1 ones col, padded to 5*128)
KES = 6                   # M3 contraction subtiles: KE rows + ones/b1 row, padded to 768
CHID = HID // 2           # classifier hidden (2048)
MSUB = S // 128           # 4 m-subtiles per 512-node core slice
NAG = 2                   # h1 column halves, one AllGather each
HQ = HID // NAG           # 2048
WQ = HID // 4             # 1024: M5 w2 ring quarter
K2S = 34                  # M5 contraction subtiles: HID + ones/b2 row, padded to 4352

BF16 = mybir.dt.bfloat16
F32 = mybir.dt.float32
FP8 = mybir.dt.float8e4
ADT = FP8

WSCALE = 64.0             # W2 pre-scale (divided out of hm)
CBS = 2048.0              # wcb pre-scale (divided out in the M3 eviction)

_COMPILED = {}


def _build_graph(use_b2=False):
    nc = bacc.Bacc(
        "TRN2",
        target_bir_lowering=False,
        debug=False,
        enable_asserts=False,
        num_devices=NCORES,
    )

    # ---- kernel I/O (per core) ----
    xe = nc.dram_tensor("xe", [N, KE], ADT, kind="ExternalInput")           # x_ext (replicated)
    at8 = nc.dram_tensor("at8", [N, S], ADT, kind="ExternalInput")          # A.T[:, rows_c] (sharded)
    wcb = nc.dram_tensor("wcb", [KES * 128, HID], ADT, kind="ExternalInput")  # CBS*(We_ext.T W1.T; b1) (repl)
    w2 = nc.dram_tensor("w2", [K2S * 128, HID], ADT, kind="ExternalInput")  # WSCALE*(W2.T; b2) (repl)
    wc1 = nc.dram_tensor("wc1", [S, CHID], BF16, kind="ExternalInput")      # Wc1.T row-chunk (sharded)
    bc1 = nc.dram_tensor("bc1", [128, CHID // 128], F32, kind="ExternalInput")  # bc1 [128,16]
    wc2 = nc.dram_tensor("wc2", [128, CHID // 128], F32, kind="ExternalInput")  # Wc2 [128,16]
    res = nc.dram_tensor("res", [1, 1], F32, kind="ExternalOutput")         # final scalar (pre-bc2)

    # ---- internal DRAM ----
    h1c_q = [nc.dram_tensor(f"h1c{i}", [S, HQ], ADT) for i in range(NAG)]
    h1f_q = [
        nc.dram_tensor(f"h1f{i}", [N, HQ], ADT, addr_space="Shared")
        for i in range(NAG)
    ]
    zb = nc.dram_tensor("zb", [1, CHID], F32)           # local partial Wc1 @ hm
    zf = nc.dram_tensor("zf", [1, CHID], F32, addr_space="Shared")  # allreduced

    NT = HID // 512   # 8 n-tiles of 512
    CI = CHID // 128  # 16
    DR = mybir.MatmulPerfMode.DoubleRow
    Relu = mybir.ActivationFunctionType.Relu
    Copy = mybir.ActivationFunctionType.Copy
    groups = [list(range(NCORES))]

    def scalar_copy_reducer(nc_, psum, sbuf, md):
        # pinned to the scalar engine: 'any'-assigned PSUM readers sometimes
        # land on the vector engine, which was measured to slow concurrent
        # PE matmuls by ~15%
        nc_.scalar.activation(out=sbuf, in_=psum, func=Copy)

    w2v = w2[:, :].rearrange("(po pi) n -> pi po n", pi=128)
    h1fv = [h1f_q[i][:, :].rearrange("(po pi) n -> pi po n", pi=128) for i in range(NAG)]

    with tile.TileContext(nc) as tc:
        with contextlib.ExitStack() as octx:
            const = octx.enter_context(tc.tile_pool(name="const", bufs=1))
            hm_parts = const.tile([128, MSUB, NT], F32, name="hm_parts")
            hm_f32 = const.tile([128, MSUB], F32, name="hm_f32")
            hm16 = const.tile([128, MSUB], BF16, name="hm16")

            head = octx.enter_context(tc.tile_pool(name="head", bufs=1))
            wc1_t = head.tile([128, MSUB, CHID], BF16, name="wc1_t")
            bc1_t = head.tile([128, CI], F32, name="bc1_t")
            wc2_t = head.tile([128, CI], F32, name="wc2_t")
            zp_t = head.tile([1, CHID], F32, name="zp_t")
            z2_t = head.tile([128, CI], F32, name="z2_t")
            zcol_t = head.tile([128, 1], F32, name="zcol_t")
            ones_t = head.tile([128, 1], F32, name="ones_t")
            r_t = head.tile([1, 1], F32, name="r_t")
            zero_t = head.tile([128, 512], F32, name="zero_t")
            nc.any.memset(ones_t[:], 1.0)
            nc.any.memset(zero_t[:], 0.0)

            # persistent SBUF caches for chained-matmul intermediates
            cache = octx.enter_context(tc.tile_pool(name="cache", bufs=1))
            at8_c = cache.tile([128, N // 128, S], ADT, name="at8_c")
            tT_c = cache.tile([128, KES, S], ADT, name="tT_c")
            wcb_c = cache.tile([128, KES, HID], ADT, name="wcb_c")
            a2T_c = cache.tile([128, K2S, S], ADT, name="a2T_c")
            # w2 ring: 2 quarters x 2 halves as separate 512-wide tiles so
            # the matmul rhs pattern is stride-512 (measured ~15% faster on
            # the PE than 2048-wide strided reads)
            w2t = [cache.tile([128, K2S, 512], ADT, name=f"w2t{i}") for i in range(4)]

            nc.any.memset(hm_parts[:], 0.0)
            # ones/bias contraction rows: tT row 640 multiplies wcb's b1 row;
            # a2T row 4096 multiplies w2's b2 row (only when b2 is nonzero)
            nc.any.memset(tT_c[:, KES - 1 :, :], 0.0)
            nc.any.memset(tT_c[0:1, KES - 1 :, :], 1.0)
            nc.any.memset(a2T_c[:, 32:, :], 0.0)
            if use_b2:
                nc.any.memset(a2T_c[0:1, 32:33, :], 1.0)

            # at8 cache: chunk 0 up front; M1's kxn producer pulls in the
            # rest per k-tile so M1's xe stream is never queued behind it
            at8v = at8[:, :].rearrange("(po pi) n -> pi po n", pi=128)
            nc.sync.dma_start(out=at8_c[:, 0:8, :], in_=at8v[:, 0:8, :])
            wcbv = wcb[:, :].rearrange("(po pi) n -> pi po n", pi=128)

            noop = lambda nc_, sbuf, md: None

            # PE warm-up: ~24 junk matmuls on a zeroed tile fill the
            # DMA-wait window before M1's inputs land and pre-ramp the
            # power state (M1 was measured at ~390ns/matmul in the ramp
            # vs 219ns warm)
            with contextlib.ExitStack() as wctx:
                wpool = wctx.enter_context(tc.tile_pool(name="warm", bufs=1))
                wpsum = wctx.enter_context(
                    tc.tile_pool(name="wpsum", bufs=1, space="PSUM")
                )
                warm_t = wpool.tile([128, 2, 512], ADT, name="warm_t")
                wscr = wpool.tile([128, 512], F32, name="wscr")
                wps = wpsum.tile([128, 512], F32, name="wps")
                nc.any.memset(warm_t[:], 0.0)
                for i in range(24):
                    nc.tensor.matmul(
                        wps[:, :],
                        warm_t[:, :, 0:128],
                        warm_t[:, :, :],
                        start=(i == 0),
                        stop=(i == 23),
                        perf_mode=DR,
                    )
                nc.scalar.activation(out=wscr[:, :], in_=wps[:, :], func=Copy)

            # M1: tT = (A_c @ x_ext).T = xe.T @ A_c.T        [KE, S] fp8
            m1_loaded = {0}

            def m1_kxn(nc_, md):
                kt = md.k_tile_idx
                if kt not in m1_loaded:
                    m1_loaded.add(kt)
                    nc_.sync.dma_start(
                        out=at8_c[:, ts(kt, md.k_subtiles), :],
                        in_=at8v[:, ts(kt, md.k_subtiles), :],
                    )
                return at8_c[:, ts(kt, md.k_subtiles), :]

            with contextlib.ExitStack() as ctx:
                kxm_pool = ctx.enter_context(tc.tile_pool(name="m1kxm", bufs=10))
                kxm_producer, kxm_shape = dma_from_dram_kxm(kxm_pool, xe[:, :])
                composable_matmul_tile_kernel(
                    tc=tc,
                    kxm_shape=kxm_shape,
                    kxn_shape=ShapeInfo(pdims=((128, N // 128),), fdims=(S,)),
                    output_type=None,
                    kxm_producer=kxm_producer,
                    kxn_producer=m1_kxn,
                    mxn_subtile_reducer=scalar_copy_reducer,
                    mxn_consumer=noop,
                    mxn_subtile_producer=lambda nc_, md: tT_c[
                        :, md.m_tile_idx : md.m_tile_idx + 1, :
                    ],
                    psum_n_bufs=2,
                    MAX_K_TILE_SIZE=1024,
                )
            # wcb half 0 streams in during M1's compute; half 1 is issued
            # inside the M3 loop behind the first AllGather trigger
            nc.sync.dma_start(out=wcb_c[:, :, 0:HQ], in_=wcbv[:, :, 0:HQ])

            # M3: h1 = relu((tT.T @ wcb) / CBS)   [S, HID] fp8, manual loop.
            # Column-quarter-major so each AllGather fires as soon as its
            # quarter is complete; each DoubleRow stationary feeds 2 PSUM
            # banks; 4 bank-pairs cycle so evictions never stall the PE.
            with contextlib.ExitStack() as ctx:
                m3psum = ctx.enter_context(
                    tc.tile_pool(name="m3psum", bufs=1, space="PSUM")
                )
                m3ps = [
                    [m3psum.tile([128, 512], F32, name=f"m3ps{p}_{b}") for b in range(4)]
                    for p in range(2)
                ]
                m3out = ctx.enter_context(tc.tile_pool(name="m3out", bufs=4))
                for q in range(NAG):
                    for ms in range(MSUB):
                        grp = m3ps[(q * MSUB + ms) % 2]
                        for kk in range(KES // 2):
                            lhsT = tT_c[:, 2 * kk : 2 * kk + 2, 128 * ms : 128 * (ms + 1)]
                            for b in range(4):
                                nc.tensor.matmul(
                                    grp[b][:, :],
                                    lhsT,
                                    wcb_c[:, 2 * kk : 2 * kk + 2,
                                          HQ * q + 512 * b : HQ * q + 512 * (b + 1)],
                                    start=(kk == 0),
                                    stop=(kk == KES // 2 - 1),
                                    perf_mode=DR,
                                )
                        h1t = m3out.tile([128, HQ], ADT, tag="h1t")
                        for b in range(4):
                            # alternate scalar/vector so neither engine lags
                            if b % 2 == 0:
                                nc.scalar.activation(
                                    out=h1t[:, 512 * b : 512 * (b + 1)],
                                    in_=grp[b][:, :], func=Relu, scale=1.0 / CBS,
                                )
                            else:
                                nc.vector.tensor_scalar(
                                    h1t[:, 512 * b : 512 * (b + 1)], grp[b][:, :],
                                    1.0 / CBS, 0.0,
                                    mybir.AluOpType.mult, mybir.AluOpType.max,
                                )
                        nc.sync.dma_start(
                            out=h1c_q[q][128 * ms : 128 * (ms + 1), :], in_=h1t[:, :]
                        )
                    nc.gpsimd.collective_compute(
                        "AllGather",
                        mybir.AluOpType.bypass,
                        ins=[h1c_q[q][:, :].opt()],
                        outs=[h1f_q[q][:, :].opt()],
                        replica_groups=groups,
                    )
                    if q == 0:
                        # wcb half 1 rides behind half 0's h1c writes so the
                        # first AllGather trigger isn't queued behind it
                        nc.sync.dma_start(
                            out=wcb_c[:, :, HQ : 2 * HQ], in_=wcbv[:, :, HQ : 2 * HQ]
                        )

            # prefetches for M5/M6 fill the DMA-idle window while the PE waits
            # on the first AllGather: w2 quarter 0, then the head weights
            for b in range(2):
                nc.sync.dma_start(
                    out=w2t[b][:, :, :], in_=w2v[:, :, 512 * b : 512 * (b + 1)]
                )
            nc.sync.dma_start(
                out=wc1_t[:, :, :],
                in_=wc1[:, :].rearrange("(po pi) n -> pi po n", pi=128),
            )
            nc.sync.dma_start(out=bc1_t[:, :], in_=bc1[:, :])
            nc.sync.dma_start(out=wc2_t[:, :], in_=wc2[:, :])

            # M4: a2T = (A_c @ h1).T = h1f.T @ A_c.T         [HID, S] fp8.
            # One composable matmul over all of HID; the kxm producer sources
            # each m-tile from its gathered quarter tensor.
            with contextlib.ExitStack() as ctx:
                m4pool = ctx.enter_context(tc.tile_pool(name="m4kxm", bufs=8))

                def m4_kxm(nc_, md):
                    t = m4pool.tile([128, md.k_subtiles, 512], ADT, tag="m4kxm")
                    q, loc = divmod(md.m_tile_idx, HQ // 512)
                    nc_.sync.dma_start(
                        t[:],
                        h1fv[q][:, ts(md.k_tile_idx, md.k_subtiles), ds(loc * 512, 512)],
                    )
                    return t

                kxn_producer, kxn_shape = dma_from_dram_kxn(
                    None, at8_c[:, :, :], kxn_cache=at8_c[:, :, :]
                )
                composable_matmul_tile_kernel(
                    tc=tc,
                    kxm_shape=ShapeInfo(pdims=((128, N // 128),), fdims=(HID,)),
                    kxn_shape=kxn_shape,
                    output_type=None,
                    kxm_producer=m4_kxm,
                    kxn_producer=kxn_producer,
                    mxn_subtile_reducer=scalar_copy_reducer,
                    mxn_consumer=noop,
                    mxn_subtile_producer=lambda nc_, md: a2T_c[
                        :, 4 * md.m_tile_idx : 4 * md.m_tile_idx + 4, :
                    ],
                    psum_n_bufs=2,
                    MAX_K_TILE_SIZE=1024,
                )

            # M5: h2 = relu(a2T.T @ w2); only row-sums survive (-> hm_parts).
            # Manual loop: w2 streams through a 2-quarter SBUF ring, each
            # DoubleRow stationary feeds 2 banks, relu+row-sum fused into the
            # scalar-engine eviction.  b2 rides as contraction row 4096.
            KP = K2S // 2 if use_b2 else 16
            with contextlib.ExitStack() as ctx:
                m5psum = ctx.enter_context(
                    tc.tile_pool(name="m5psum", bufs=1, space="PSUM")
                )
                m5ps = [
                    [m5psum.tile([128, 512], F32, name=f"m5ps{p}_{b}") for b in range(2)]
                    for p in range(4)
                ]
                m5scr = ctx.enter_context(tc.tile_pool(name="m5scr", bufs=2))
                for q in range(4):
                    if q + 1 < 4:  # double-buffer the next w2 quarter
                        for b in range(2):
                            sl_n = 2 * ((q + 1) % 2) + b
                            nc.sync.dma_start(
                                out=w2t[sl_n][:, :, :],
                                in_=w2v[:, :, WQ * (q + 1) + 512 * b :
                                        WQ * (q + 1) + 512 * (b + 1)],
                            )
                    for ms in range(MSUB):
                        pair = m5ps[(q * MSUB + ms) % 4]
                        for b in range(2):
                            rhs_t = w2t[2 * (q % 2) + b]
                            for kk in range(KP):
                                nc.tensor.matmul(
                                    pair[b][:, :],
                                    a2T_c[:, 2 * kk : 2 * kk + 2,
                                          128 * ms : 128 * (ms + 1)],
                                    rhs_t[:, 2 * kk : 2 * kk + 2, :],
                                    start=(kk == 0),
                                    stop=(kk == KP - 1),
                                    perf_mode=DR,
                                )
                            # evict via scalar relu into SBUF, then row-sum on
                            # the vector engine from SBUF: keeps both the HW
                            # accumulator and vector-engine PSUM reads out of
                            # the PE's way (each was measured to slow
                            # concurrent matmuls by ~15%)
                            scr = m5scr.tile([128, 512], F32, tag="m5scr")
                            nc.scalar.activation(
                                out=scr[:, :], in_=pair[b][:, :], func=Relu
                            )
                            nc.vector.tensor_reduce(
                                out=hm_parts[:, ms, 2 * q + b : 2 * q + b + 1],
                                in_=scr[:, :],
                                axis=mybir.AxisListType.X, op=mybir.AluOpType.add,
                            )

            # hm_c = rowsum(h2_c) / (HID * WSCALE), kept in SBUF (bf16 for M6)
            nc.vector.tensor_reduce(
                out=hm_f32[:, :], in_=hm_parts[:, :, :],
                axis=mybir.AxisListType.X, op=mybir.AluOpType.add,
            )
            nc.vector.tensor_scalar_mul(hm16[:, :], hm_f32[:, :], 1.0 / (HID * WSCALE))

            # M6 (head): zp = Wc1[:, local] @ hm_local, AllReduce over cores,
            # then z = relu(z + bc1), res = z . Wc2.  Bank-interleaved so the
            # 16 tiny matmuls pipeline instead of serializing per bank.
            hpsum = octx.enter_context(tc.tile_pool(name="hpsum", bufs=1, space="PSUM"))
            NB = CHID // 512  # 4 psum banks
            ps6 = [hpsum.tile([128, 512], F32, name=f"ps{j}") for j in range(NB)]
            psr = hpsum.tile([128, 512], F32, name="psr")
            for ko in range(MSUB):
                for j in range(NB):
                    nc.tensor.matmul(
                        ps6[j][0:1, :],
                        hm16[:, ko : ko + 1],
                        wc1_t[:, ko, 512 * j : 512 * (j + 1)],
                        start=(ko == 0),
                        stop=(ko == MSUB - 1),
                    )
            for j in range(NB):
                nc.vector.tensor_copy(
                    out=zp_t[:, 512 * j : 512 * (j + 1)], in_=ps6[j][0:1, :]
                )
            nc.sync.dma_start(out=zb[:, :], in_=zp_t[:, :])
            nc.gpsimd.collective_compute(
                "AllReduce",
                mybir.AluOpType.add,
                ins=[zb[:, :].opt()],
                outs=[zf[:, :].opt()],
                replica_groups=groups,
            )
            # epilogue on z viewed as [128, 16] so the DVE ops use all lanes
            nc.sync.dma_start(
                out=z2_t[:, :], in_=zf[:, :].rearrange("o (p i) -> p (o i)", p=128)
            )
            nc.vector.tensor_add(out=z2_t[:, :], in0=z2_t[:, :], in1=bc1_t[:, :])
            # fused relu * wc2 with row-sum: out = max(z2, 0) * wc2,
            # zcol = sum(out) — one DVE op instead of three
            nc.vector.scalar_tensor_tensor(
                out=z2_t[:, :], in0=z2_t[:, :], scalar=0.0, in1=wc2_t[:, :],
                op0=mybir.AluOpType.max, op1=mybir.AluOpType.mult,
                accum_out=zcol_t[:, :],
            )
            # cross-partition sum via a 128x1 ones matmul
            nc.tensor.matmul(
                psr[0:1, 0:1], ones_t[:, 0:1], zcol_t[:, 0:1], start=True, stop=True
            )
            nc.vector.tensor_copy(out=r_t[:, :], in_=psr[0:1, 0:1])
            nc.sync.dma_start(out=res[:, :], in_=r_t[:, :])

    nc.compile()
    nc.m = get_hw_module(nc.m)
    return nc


def get_compiled(use_b2=False):
    key = bool(use_b2)
    if key not in _COMPILED:
        _COMPILED[key] = _build_graph(key)
    return _COMPILED[key]


def _f32(a):
    return np.ascontiguousarray(np.asarray(a, dtype=np.float32))


_NP_FP8 = mybir.dt.np(FP8)


def _adt(a):
    a = np.ascontiguousarray(np.asarray(a, dtype=np.float32))
    return np.clip(a, -240.0, 240.0).astype(_NP_FP8)


def make_in_maps(x, edge_index, W_embed, b_embed, W1, b1, W2, b2, Wc1, bc1, Wc2, bc2):
    x = _f32(x)
    ei = np.asarray(edge_index).astype(np.int64)
    # adjacency counts, transposed: AT[src, dst] = #edges src->dst
    counts = np.bincount(ei[1] * N + ei[0], minlength=N * N).astype(np.float32)
    AT = counts.reshape(N, N)

    x_ext = np.zeros((N, KE), np.float32)
    x_ext[:, :IN_DIM] = x
    x_ext[:, IN_DIM] = 1.0

    we_ext = np.zeros((KE, HID), np.float32)
    we_ext[:IN_DIM] = _f32(W_embed).T
    we_ext[IN_DIM] = _f32(b_embed)
    # layer-1 transform is low-rank: fold We_ext.T @ W1.T on the host.
    # Row 640 holds b1, matched by the all-ones tT contraction row.
    wcb_full = np.zeros((KES * 128, HID), np.float32)
    wcb_full[:KE] = we_ext @ _f32(W1).T
    wcb_full[KE] = _f32(b1)
    wcb_np = _adt(wcb_full * CBS)

    w2p = np.zeros((K2S * 128, HID), np.float32)
    w2p[:HID] = _f32(W2).T * WSCALE
    w2p[HID] = _f32(b2) * WSCALE
    w2_np = _adt(w2p)

    xe_np = _adt(x_ext)
    at8_np = _adt(AT)
    wc1T = _f32(Wc1).T.astype(ml_dtypes.bfloat16)  # [HID(nodes), CHID]
    wc2_row = _f32(Wc2).reshape(128, CHID // 128)
    bc1_full = _f32(bc1).reshape(128, CHID // 128)

    in_maps = []
    for c in range(NCORES):
        rows = slice(S * c, S * (c + 1))
        in_maps.append(
            {
                "xe": xe_np,
                "wcb": wcb_np,
                "at8": np.ascontiguousarray(at8_np[:, rows]),
                "w2": w2_np,
                "wc1": np.ascontiguousarray(wc1T[rows, :]),
                "bc1": bc1_full,
                "wc2": wc2_row,
            }
        )
    return in_maps


def kernel(**inputs):
    use_b2 = bool(np.any(np.asarray(inputs["b2"], dtype=np.float32)))
    nc = get_compiled(use_b2)
    in_maps = make_in_maps(**inputs)
    bres = run_bass_kernel_spmd(nc, in_maps, core_ids=list(range(NCORES)))
    val = np.float32(bres.results[0]["res"][0, 0])
    bc2 = np.asarray(inputs["bc2"], dtype=np.float32).reshape(-1)
    out = np.asarray(val + bc2[0], dtype=np.float32).reshape(())
    return out
